# revision 12
# baseline (speedup 1.0000x reference)
"""Multi-head attention block on 8 Trainium2 NeuronCores, data-parallel over
batch, fp8 (e4m3) matmul datapath with DoubleRow perf mode.

Shapes (hardcoded): B=8, S=1024, H=16, HD=64, D=1024. One batch element per
core. Host pre-transposes/casts x and the weights to fp8 (weights scaled by
64 for fp8 range; the 1/64 is folded into the PSUM movers), compacts keys
(unmasked first), and applies gamma/beta after gathering (they are identity
for the spec inputs). Biases are zero per the input spec and asserted so.

Per-core dataflow:
  phase 1: qT/kT (fp8, [128d-of-dt, DT, S|SC]) and v' ([128keys, kc, H, 72]
           with a ones column per head at col 64) via DoubleRow fp8 matmuls
           over host-packed xT8/xcT8/w8; DVE tensor_scalar movers apply 1/64.
  phase 2 per head: scoresT[k,q] psum <- kT-slice^T @ qT (K=64);
           E = exp(0.125*scores + maskneg_k) via one ACT per kt -> fp8 SBUF;
           ctx' psum [65, 2, 512] accumulates v'^T @ E DoubleRow over kt
           pairs (rows 0..63 ctx, row 64 = colsum = softmax denominator);
           denom reciprocal via one int32 Schraudolph tensor_scalar on the
           psum row, gpsimd partition_broadcast, then one fused TT that
           normalizes and casts ctx -> fp8 ctxT.
  phase 3: out proj fp8 DoubleRow (ctxT @ wo8), epilogue
           t = x + proj/64, LayerNorm via bn_stats/bn_aggr + sqrt + recip.
"""
import sys
import time

sys.path.insert(0, "/opt/trn_rl_repo")

import numpy as np
import ml_dtypes

import concourse.bass as bass
import concourse.bacc as bacc
import concourse.tile as tile
from concourse import mybir
from concourse.bass_utils import run_bass_kernel_spmd

F32 = mybir.dt.float32
FP8 = mybir.dt.float8e4
INT32 = mybir.dt.int32
AF = mybir.ActivationFunctionType
DR = mybir.MatmulPerfMode.DoubleRow
MUL = mybir.AluOpType.mult
ADD = mybir.AluOpType.add

B, S, H, HD = 8, 1024, 16, 64
D = H * HD
NINF = -10000.0
EPS = 1e-6
ST = S // 128    # 8 s-tiles
DT = D // 128    # 8 d-tiles
NH = S // 512    # 2 query halves
WS = 64.0        # weight scale for fp8
IWS = 1.0 / WS
VST = 72         # per-head stride in v' (64 v dims + ones col + pad)
RECIP_C = 0x7EF311C3  # Schraudolph reciprocal constant


def _emit_body(nc, tc, io, cst, kc):
    (x32, out) = io
    (mneg_sb, eps_sb, qT, kT, vp, ctxT, xT_sb, xcT_sb,
     wq_sb, wk_sb, wv_sb, wo_sb, x_sb) = cst
    SC = kc * 128
    kpairs = kc // 2          # full DoubleRow key-tile pairs for AV
    ktail = kc - 2 * kpairs   # 0 or 1 leftover key tile

    if True:
        # ---------------- phases 1+2 merged: projections feed heads ------
        # V first, then per dt: Q/K projections for that d-group followed by
        # its two heads. The projection matmuls keep the PE busy during the
        # exp (ACT) waits of the attention chain, so HAM stays warm.
        with (
            tc.tile_pool(name="Ep", bufs=2) as Ep,
            tc.tile_pool(name="nrm", bufs=2) as nrm,
            tc.tile_pool(name="ps1", bufs=1, space="PSUM") as ps1,
            tc.tile_pool(name="scps", bufs=2, space="PSUM") as scps,
            tc.tile_pool(name="ctxps", bufs=3, space="PSUM") as ctxps,
        ):
            # v' natural [keys, dcols], scattered to head slots
            for st in range(kc):
                for et in range(2):
                    vps = ps1.tile([128, 512], F32, tag="qps", name="vps")
                    for kp in range(0, DT, 2):
                        nc.tensor.matmul(
                            vps,
                            xcT_sb[:, kp:kp + 2, st * 128:(st + 1) * 128],
                            wv_sb[:, kp:kp + 2, et * 512:(et + 1) * 512],
                            start=(kp == 0), stop=(kp == DT - 2),
                            perf_mode=DR)
                    nc.vector.tensor_scalar(
                        vp[:, st, et * 8:(et + 1) * 8, 0:64],
                        vps.rearrange("p (h j) -> p h j", j=64),
                        IWS, None, op0=MUL)

            k_tiles = [(i * 512, min(512, SC - i * 512))
                       for i in range((SC + 511) // 512)]
            q_tiles = [(i * 512, 512) for i in range(NH)]

            def project(wsb_t, dstT, rhsT, mg, ntiles):
                for n0, nsz in ntiles:
                    qps = ps1.tile([128, 512], F32, tag="qps", name="qps")
                    for kp in range(0, DT, 2):
                        nc.tensor.matmul(
                            qps[:, 0:nsz],
                            wsb_t[:, kp:kp + 2, mg * 128:(mg + 1) * 128],
                            rhsT[:, kp:kp + 2, n0:n0 + nsz],
                            start=(kp == 0), stop=(kp == DT - 2),
                            perf_mode=DR)
                    nc.vector.tensor_scalar(
                        dstT[:, mg, n0:n0 + nsz], qps[:, 0:nsz],
                        IWS, None, op0=MUL)

            def head(h):
                base = (h % 2) * 64
                dt = h // 2
                cps = [ctxps.tile([65, 512], F32, tag="ctx", name="cps")
                       for _ in range(NH)]
                e_pair = None
                for kt in range(kc):
                    scp = scps.tile([128, 2, 512], F32, tag="scp",
                                    name="scp")
                    for qh in range(NH):
                        nc.tensor.matmul(
                            scp[:, qh, :],
                            kT[base:base + 64, dt,
                               kt * 128:(kt + 1) * 128],
                            qT[base:base + 64, dt,
                               qh * 512:(qh + 1) * 512],
                            start=True, stop=True)
                    if kt % 2 == 0:
                        e_pair = Ep.tile([128, 2, 2, 512], FP8, tag="E",
                                         name="e_pair")
                    nc.scalar.activation(
                        e_pair[:, kt % 2, :, :], scp, AF.Exp,
                        bias=mneg_sb[:, kt:kt + 1], scale=0.125)
                    last_pair = (kt % 2 == 1 and ktail == 0
                                 and kt == kc - 1)
                    if kt % 2 == 1:
                        for qh in range(NH):
                            nc.tensor.matmul(
                                cps[qh],
                                vp[:, kt - 1:kt + 1, h, 0:65],
                                e_pair[:, 0:2, qh, :],
                                start=(kt == 1), stop=last_pair,
                                perf_mode=DR)
                    elif kt == kc - 1:
                        # odd kc tail: DoubleRow against the zeroed pad
                        # plane of vp. The moving pair dim has stride 0 so
                        # plane 1 re-reads the freshly written plane 0 --
                        # never stale SBUF (uninitialized e4m3 can be NaN,
                        # and 0 * NaN = NaN even against zero weights).
                        for qh in range(NH):
                            e0 = e_pair[:, 0, qh, :]
                            e00 = bass.AP(
                                tensor=e0.tensor, offset=e0.offset,
                                ap=[list(e0.ap[0]), [0, 2]]
                                + [list(a) for a in e0.ap[1:]])
                            nc.tensor.matmul(
                                cps[qh],
                                vp[:, kt:kt + 2, h, 0:65],
                                e00,
                                start=(kc == 1), stop=True,
                                perf_mode=DR)

                # softmax denominators: Schraudolph reciprocal of psum row 64
                for qh in range(NH):
                    r0 = nrm.tile([1, 512], F32, tag=f"r0{qh}", name="r0")
                    nc.vector.tensor_scalar(
                        r0.bitcast(INT32), cps[qh][64:65, :].bitcast(INT32),
                        -1, RECIP_C, op0=MUL, op1=ADD)
                    rbc = nrm.tile([64, 512], F32, tag=f"rbc{qh}",
                                   name="rbc")
                    nc.gpsimd.partition_broadcast(rbc, r0)
                    # fused normalize + psum drain + fp8 cast
                    nc.vector.tensor_mul(
                        ctxT[base:base + 64, dt,
                             qh * 512:(qh + 1) * 512],
                        cps[qh][0:64, :], rbc)

            for dt in range(DT):
                project(wq_sb, qT, xT_sb, dt, q_tiles)
                project(wk_sb, kT, xcT_sb, dt, k_tiles)
                head(2 * dt)
                head(2 * dt + 1)

        # ---------------- phase 3: out proj + LayerNorm ----------------
        with (
            tc.tile_pool(name="epi", bufs=2) as epi,
            tc.tile_pool(name="pjps", bufs=2, space="PSUM") as pjps,
        ):
            for qt in range(ST):
                x_t = x_sb[:, qt, :]
                t = epi.tile([128, D], F32, tag="t", name="t")
                for et in range(2):
                    pps = pjps.tile([128, 512], F32, tag=f"pj{et}",
                                    name="pps")
                    for dp in range(0, DT, 2):
                        nc.tensor.matmul(
                            pps,
                            ctxT[:, dp:dp + 2, qt * 128:(qt + 1) * 128],
                            wo_sb[:, dp:dp + 2, et * 512:(et + 1) * 512],
                            start=(dp == 0), stop=(dp == DT - 2),
                            perf_mode=DR)
                    nc.vector.scalar_tensor_tensor(
                        out=t[:, et * 512:(et + 1) * 512],
                        in0=pps, scalar=IWS,
                        in1=x_t[:, et * 512:(et + 1) * 512],
                        op0=MUL, op1=ADD)
                stats = epi.tile([128, 2, nc.vector.BN_STATS_DIM], F32,
                                 tag="stats", name="stats")
                tg = t.rearrange("p (g d) -> p g d", g=2)
                for g in range(2):
                    nc.vector.bn_stats(stats[:, g, :], tg[:, g, :])
                mv = epi.tile([128, nc.vector.BN_AGGR_DIM], F32, tag="mv",
                              name="mv")
                nc.vector.bn_aggr(mv, stats)
                rstd = epi.tile([128, 1], F32, tag="rstd", name="rstd")
                nc.scalar.activation(
                    rstd, mv[:, 1:2], AF.Sqrt, bias=eps_sb, scale=1.0)
                nc.vector.reciprocal(rstd, rstd)
                nmr = epi.tile([128, 1], F32, tag="nmr", name="nmr")
                nc.vector.tensor_scalar(
                    nmr, mv[:, 0:1], rstd, -1.0, op0=MUL, op1=MUL)
                o_t = epi.tile([128, D], F32, tag="ot", name="o_t")
                nc.scalar.activation(
                    o_t, t, AF.Identity, bias=nmr, scale=rstd)
                nc.sync.dma_start(
                    out=out[qt * 128:(qt + 1) * 128, :], in_=o_t)


def build_bass(reps=1, kc=8):
    nc = bacc.Bacc("TRN2", target_bir_lowering=False, debug=False)

    SC = kc * 128
    x32 = nc.dram_tensor("x32", [S, D], F32, kind="ExternalInput").ap()
    xT8 = nc.dram_tensor("xT8", [128, DT, S], FP8, kind="ExternalInput").ap()
    xcT8 = nc.dram_tensor("xcT8", [128, DT, SC], FP8,
                          kind="ExternalInput").ap()
    wq8 = nc.dram_tensor("wq8", [128, DT, D], FP8, kind="ExternalInput").ap()
    wk8 = nc.dram_tensor("wk8", [128, DT, D], FP8, kind="ExternalInput").ap()
    wv8 = nc.dram_tensor("wv8", [128, DT, D], FP8, kind="ExternalInput").ap()
    wo8 = nc.dram_tensor("wo8", [128, DT, D], FP8, kind="ExternalInput").ap()
    maskneg = nc.dram_tensor("maskneg", [SC], F32, kind="ExternalInput").ap()
    out = nc.dram_tensor("out", [S, D], F32, kind="ExternalOutput").ap()
    io = (x32, out)

    with tile.TileContext(nc) as tc:
        with tc.tile_pool(name="const", bufs=1) as const:
            mneg_sb = const.tile([128, kc], F32, name="mneg_sb")
            nc.sync.dma_start(out=mneg_sb,
                              in_=maskneg.rearrange("(t p) -> p t", p=128))
            eps_sb = const.tile([128, 1], F32, name="eps_sb")
            nc.vector.memset(eps_sb, EPS)
            # invocation-constant inputs: loaded once, resident in SBUF
            xT_sb = const.tile([128, DT, S], FP8, name="xT_sb")
            nc.sync.dma_start(out=xT_sb, in_=xT8)
            xcT_sb = const.tile([128, DT, SC], FP8, name="xcT_sb")
            nc.sync.dma_start(out=xcT_sb, in_=xcT8)
            wq_sb = const.tile([128, DT, D], FP8, name="wq_sb")
            nc.sync.dma_start(out=wq_sb, in_=wq8)
            wk_sb = const.tile([128, DT, D], FP8, name="wk_sb")
            nc.sync.dma_start(out=wk_sb, in_=wk8)
            wv_sb = const.tile([128, DT, D], FP8, name="wv_sb")
            nc.sync.dma_start(out=wv_sb, in_=wv8)
            wo_sb = const.tile([128, DT, D], FP8, name="wo_sb")
            nc.sync.dma_start(out=wo_sb, in_=wo8)
            x_sb = const.tile([128, ST, D], F32, name="x_sb")
            nc.sync.dma_start(out=x_sb,
                              in_=x32.rearrange("(t p) d -> p t d", p=128))
            qT = const.tile([128, DT, S], FP8, name="qT")
            kT = const.tile([128, DT, SC], FP8, name="kT")
            vp = const.tile([128, kc + 1, H, VST], FP8, name="vp")
            ctxT = const.tile([128, DT, S], FP8, name="ctxT")
            # ones columns of v' (col 64 of each head slot): body movers only
            # touch cols 0..63, so write these once
            nc.vector.memset(vp.rearrange("p t h j -> p (t h) j")[:, :, 64],
                             1.0)
            # zeroed pad key-plane (must come after the ones memset): lets
            # the odd-kc AV tail run DoubleRow with a garbage E plane
            nc.vector.memset(vp[:, kc, :, :], 0.0)
            cst = (mneg_sb, eps_sb, qT, kT, vp, ctxT, xT_sb, xcT_sb,
                   wq_sb, wk_sb, wv_sb, wo_sb, x_sb)
            for _ in range(reps):
                _emit_body(nc, tc, io, cst, kc)

    nc.compile()
    return nc


_NC_CACHE = {}


def _get_nc(reps=1, kc=8):
    if (reps, kc) not in _NC_CACHE:
        _NC_CACHE[(reps, kc)] = build_bass(reps, kc)
    return _NC_CACHE[(reps, kc)]


def _pack_w(w):
    # [D, D] -> [128, DT, D] fp8 with w8[p, t, n] = w[t*128+p, n] * WS
    return np.ascontiguousarray(
        (np.asarray(w, np.float32) * WS).reshape(DT, 128, D)
        .transpose(1, 0, 2)).astype(ml_dtypes.float8_e4m3)


def _pack_xT(x):
    # [S', D] -> [128, DT, S'] fp8 with xT8[p, t, s] = x[s, t*128+p]
    return np.ascontiguousarray(
        np.asarray(x, np.float32).T.reshape(DT, 128, -1)
        .transpose(1, 0, 2)).astype(ml_dtypes.float8_e4m3)


def make_in_maps(x, mask, wq, bq, wk, bk, wv, bv, wo, bo, gamma, beta):
    for b in (bq, bk, bv, bo):
        assert not np.any(np.asarray(b)), "nonzero bias unsupported"
    x = np.asarray(x, dtype=np.float32)
    mask = np.asarray(mask)
    maskneg = (mask.astype(np.float32) * NINF).astype(np.float32)
    n_un = int((mask == 0).sum(axis=1).max())
    kc = min(max((n_un + 127) // 128, 2), ST)
    SC = kc * 128
    idxs = [np.argsort(mask[c], kind="stable")[:SC] for c in range(B)]
    common = {
        "wq8": _pack_w(wq), "wk8": _pack_w(wk),
        "wv8": _pack_w(wv), "wo8": _pack_w(wo),
    }
    maps = []
    for c in range(B):
        xc = x[c][idxs[c]]
        maps.append(dict(
            common,
            x32=np.ascontiguousarray(x[c]),
            xT8=_pack_xT(x[c]),
            xcT8=_pack_xT(xc),
            maskneg=np.ascontiguousarray(maskneg[c][idxs[c]])))
    return maps, kc


def kernel(x, mask, wq, bq, wk, bk, wv, bv, wo, bo, gamma, beta):
    in_maps, kc = make_in_maps(x, mask, wq, bq, wk, bk, wv, bv, wo, bo,
                               gamma, beta)
    nc = _get_nc(1, kc)
    last_err = None
    for _ in range(3):
        try:
            res = run_bass_kernel_spmd(nc, in_maps, core_ids=list(range(B)))
            outv = np.stack([res.results[c]["out"] for c in range(B)], axis=0)
            gamma = np.asarray(gamma, np.float32)
            beta = np.asarray(beta, np.float32)
            if not (np.all(gamma == 1.0) and np.all(beta == 0.0)):
                outv = outv * gamma + beta
            return outv
        except Exception as e:  # transient NRT device errors: retry
            last_err = e
            time.sleep(5)
    raise last_err


# revision 13
# speedup vs baseline: 1.0128x; 1.0128x over previous
"""Multi-head attention block on 8 Trainium2 NeuronCores, data-parallel over
batch, fp8 (e4m3) matmul datapath with DoubleRow perf mode.

Shapes (hardcoded): B=8, S=1024, H=16, HD=64, D=1024. One batch element per
core. Host pre-transposes/casts x and the weights to fp8 (weights scaled by
64 for fp8 range; the 1/64 is folded into the PSUM movers), compacts keys
(unmasked first), and applies gamma/beta after gathering (they are identity
for the spec inputs). Biases are zero per the input spec and asserted so.

Per-core dataflow:
  phase 1: qT/kT (fp8, [128d-of-dt, DT, S|SC]) and v' ([128keys, kc, H, 72]
           with a ones column per head at col 64) via DoubleRow fp8 matmuls
           over host-packed xT8/xcT8/w8; DVE tensor_scalar movers apply 1/64.
  phase 2 per head: scoresT[k,q] psum <- kT-slice^T @ qT (K=64);
           E = exp(0.125*scores + maskneg_k) via one ACT per kt -> fp8 SBUF;
           ctx' psum [65, 2, 512] accumulates v'^T @ E DoubleRow over kt
           pairs (rows 0..63 ctx, row 64 = colsum = softmax denominator);
           denom reciprocal via one int32 Schraudolph tensor_scalar on the
           psum row, gpsimd partition_broadcast, then one fused TT that
           normalizes and casts ctx -> fp8 ctxT.
  phase 3: out proj fp8 DoubleRow (ctxT @ wo8), epilogue
           t = x + proj/64, LayerNorm via bn_stats/bn_aggr + sqrt + recip.
"""
import sys
import time

sys.path.insert(0, "/opt/trn_rl_repo")

import numpy as np
import ml_dtypes

import concourse.bass as bass
import concourse.bacc as bacc
import concourse.tile as tile
from concourse import mybir
from concourse.bass_utils import run_bass_kernel_spmd

F32 = mybir.dt.float32
FP8 = mybir.dt.float8e4
INT32 = mybir.dt.int32
AF = mybir.ActivationFunctionType
DR = mybir.MatmulPerfMode.DoubleRow
MUL = mybir.AluOpType.mult
ADD = mybir.AluOpType.add

B, S, H, HD = 8, 1024, 16, 64
D = H * HD
NINF = -10000.0
EPS = 1e-6
ST = S // 128    # 8 s-tiles
DT = D // 128    # 8 d-tiles
NH = S // 512    # 2 query halves
WS = 64.0        # weight scale for fp8
IWS = 1.0 / WS
VST = 72         # per-head stride in v' (64 v dims + ones col + pad)
RECIP_C = 0x7EF311C3  # Schraudolph reciprocal constant


def _emit_body(nc, tc, io, cst, kc):
    (x32, out) = io
    (mneg_sb, eps_sb, qT, kT, vp, ctxT, xT_sb, xcT_sb,
     wq_sb, wk_sb, wv_sb, wo_sb, x_sb) = cst
    SC = kc * 128
    kpairs = kc // 2          # full DoubleRow key-tile pairs for AV
    ktail = kc - 2 * kpairs   # 0 or 1 leftover key tile

    if True:
        # ---------------- phases 1+2 merged: projections feed heads ------
        # V first, then per dt: Q/K projections for that d-group followed by
        # its two heads. The projection matmuls keep the PE busy during the
        # exp (ACT) waits of the attention chain, so HAM stays warm.
        with (
            tc.tile_pool(name="Ep", bufs=3) as Ep,
            tc.tile_pool(name="nrm", bufs=3) as nrm,
            tc.tile_pool(name="ps1", bufs=1, space="PSUM") as ps1,
            tc.tile_pool(name="scps", bufs=2, space="PSUM") as scps,
            tc.tile_pool(name="ctxps", bufs=3, space="PSUM") as ctxps,
        ):
            # v' natural [keys, dcols], scattered to head slots
            for st in range(kc):
                for et in range(2):
                    vps = ps1.tile([128, 512], F32, tag="qps", name="vps")
                    for kp in range(0, DT, 2):
                        nc.tensor.matmul(
                            vps,
                            xcT_sb[:, kp:kp + 2, st * 128:(st + 1) * 128],
                            wv_sb[:, kp:kp + 2, et * 512:(et + 1) * 512],
                            start=(kp == 0), stop=(kp == DT - 2),
                            perf_mode=DR)
                    nc.vector.tensor_scalar(
                        vp[:, st, et * 8:(et + 1) * 8, 0:64],
                        vps.rearrange("p (h j) -> p h j", j=64),
                        IWS, None, op0=MUL)

            k_tiles = [(i * 512, min(512, SC - i * 512))
                       for i in range((SC + 511) // 512)]
            q_tiles = [(i * 512, 512) for i in range(NH)]

            def project(wsb_t, dstT, rhsT, mg, ntiles):
                for n0, nsz in ntiles:
                    qps = ps1.tile([128, 512], F32, tag="qps", name="qps")
                    for kp in range(0, DT, 2):
                        nc.tensor.matmul(
                            qps[:, 0:nsz],
                            wsb_t[:, kp:kp + 2, mg * 128:(mg + 1) * 128],
                            rhsT[:, kp:kp + 2, n0:n0 + nsz],
                            start=(kp == 0), stop=(kp == DT - 2),
                            perf_mode=DR)
                    nc.vector.tensor_scalar(
                        dstT[:, mg, n0:n0 + nsz], qps[:, 0:nsz],
                        IWS, None, op0=MUL)

            def head(h):
                base = (h % 2) * 64
                dt = h // 2
                cps = [ctxps.tile([65, 512], F32, tag="ctx", name="cps")
                       for _ in range(NH)]
                e_pair = None
                for kt in range(kc):
                    scp = scps.tile([128, 2, 512], F32, tag="scp",
                                    name="scp")
                    for qh in range(NH):
                        nc.tensor.matmul(
                            scp[:, qh, :],
                            kT[base:base + 64, dt,
                               kt * 128:(kt + 1) * 128],
                            qT[base:base + 64, dt,
                               qh * 512:(qh + 1) * 512],
                            start=True, stop=True)
                    if kt % 2 == 0:
                        e_pair = Ep.tile([128, 2, 2, 512], FP8, tag="E",
                                         name="e_pair")
                    nc.scalar.activation(
                        e_pair[:, kt % 2, :, :], scp, AF.Exp,
                        bias=mneg_sb[:, kt:kt + 1], scale=0.125)
                    last_pair = (kt % 2 == 1 and ktail == 0
                                 and kt == kc - 1)
                    if kt % 2 == 1:
                        for qh in range(NH):
                            nc.tensor.matmul(
                                cps[qh],
                                vp[:, kt - 1:kt + 1, h, 0:65],
                                e_pair[:, 0:2, qh, :],
                                start=(kt == 1), stop=last_pair,
                                perf_mode=DR)
                    elif kt == kc - 1:
                        # odd kc tail: DoubleRow against the zeroed pad
                        # plane of vp. The moving pair dim has stride 0 so
                        # plane 1 re-reads the freshly written plane 0 --
                        # never stale SBUF (uninitialized e4m3 can be NaN,
                        # and 0 * NaN = NaN even against zero weights).
                        for qh in range(NH):
                            e0 = e_pair[:, 0, qh, :]
                            e00 = bass.AP(
                                tensor=e0.tensor, offset=e0.offset,
                                ap=[list(e0.ap[0]), [0, 2]]
                                + [list(a) for a in e0.ap[1:]])
                            nc.tensor.matmul(
                                cps[qh],
                                vp[:, kt:kt + 2, h, 0:65],
                                e00,
                                start=(kc == 1), stop=True,
                                perf_mode=DR)

                # softmax denominators: Schraudolph reciprocal of psum row 64
                for qh in range(NH):
                    r0 = nrm.tile([1, 512], F32, tag=f"r0{qh}", name="r0")
                    nc.vector.tensor_scalar(
                        r0.bitcast(INT32), cps[qh][64:65, :].bitcast(INT32),
                        -1, RECIP_C, op0=MUL, op1=ADD)
                    rbc = nrm.tile([64, 512], F32, tag=f"rbc{qh}",
                                   name="rbc")
                    nc.gpsimd.partition_broadcast(rbc, r0)
                    # fused normalize + psum drain + fp8 cast
                    nc.vector.tensor_mul(
                        ctxT[base:base + 64, dt,
                             qh * 512:(qh + 1) * 512],
                        cps[qh][0:64, :], rbc)

            for dt in range(DT):
                project(wq_sb, qT, xT_sb, dt, q_tiles)
                project(wk_sb, kT, xcT_sb, dt, k_tiles)
                head(2 * dt)
                head(2 * dt + 1)

        # ---------------- phase 3: out proj + LayerNorm ----------------
        with (
            tc.tile_pool(name="epi", bufs=3) as epi,
            tc.tile_pool(name="pjps", bufs=2, space="PSUM") as pjps,
        ):
            for qt in range(ST):
                x_t = x_sb[:, qt, :]
                t = epi.tile([128, D], F32, tag="t", name="t")
                for et in range(2):
                    pps = pjps.tile([128, 512], F32, tag=f"pj{et}",
                                    name="pps")
                    for dp in range(0, DT, 2):
                        nc.tensor.matmul(
                            pps,
                            ctxT[:, dp:dp + 2, qt * 128:(qt + 1) * 128],
                            wo_sb[:, dp:dp + 2, et * 512:(et + 1) * 512],
                            start=(dp == 0), stop=(dp == DT - 2),
                            perf_mode=DR)
                    nc.vector.scalar_tensor_tensor(
                        out=t[:, et * 512:(et + 1) * 512],
                        in0=pps, scalar=IWS,
                        in1=x_t[:, et * 512:(et + 1) * 512],
                        op0=MUL, op1=ADD)
                stats = epi.tile([128, 2, nc.vector.BN_STATS_DIM], F32,
                                 tag="stats", name="stats")
                tg = t.rearrange("p (g d) -> p g d", g=2)
                for g in range(2):
                    nc.vector.bn_stats(stats[:, g, :], tg[:, g, :])
                mv = epi.tile([128, nc.vector.BN_AGGR_DIM], F32, tag="mv",
                              name="mv")
                nc.vector.bn_aggr(mv, stats)
                rstd = epi.tile([128, 1], F32, tag="rstd", name="rstd")
                nc.scalar.activation(
                    rstd, mv[:, 1:2], AF.Sqrt, bias=eps_sb, scale=1.0)
                nc.vector.reciprocal(rstd, rstd)
                nmr = epi.tile([128, 1], F32, tag="nmr", name="nmr")
                nc.vector.tensor_scalar(
                    nmr, mv[:, 0:1], rstd, -1.0, op0=MUL, op1=MUL)
                o_t = epi.tile([128, D], F32, tag="ot", name="o_t")
                nc.scalar.activation(
                    o_t, t, AF.Identity, bias=nmr, scale=rstd)
                nc.sync.dma_start(
                    out=out[qt * 128:(qt + 1) * 128, :], in_=o_t)


def build_bass(reps=1, kc=8):
    nc = bacc.Bacc("TRN2", target_bir_lowering=False, debug=False)

    SC = kc * 128
    x32 = nc.dram_tensor("x32", [S, D], F32, kind="ExternalInput").ap()
    xT8 = nc.dram_tensor("xT8", [128, DT, S], FP8, kind="ExternalInput").ap()
    xcT8 = nc.dram_tensor("xcT8", [128, DT, SC], FP8,
                          kind="ExternalInput").ap()
    wq8 = nc.dram_tensor("wq8", [128, DT, D], FP8, kind="ExternalInput").ap()
    wk8 = nc.dram_tensor("wk8", [128, DT, D], FP8, kind="ExternalInput").ap()
    wv8 = nc.dram_tensor("wv8", [128, DT, D], FP8, kind="ExternalInput").ap()
    wo8 = nc.dram_tensor("wo8", [128, DT, D], FP8, kind="ExternalInput").ap()
    maskneg = nc.dram_tensor("maskneg", [SC], F32, kind="ExternalInput").ap()
    out = nc.dram_tensor("out", [S, D], F32, kind="ExternalOutput").ap()
    io = (x32, out)

    with tile.TileContext(nc) as tc:
        with tc.tile_pool(name="const", bufs=1) as const:
            mneg_sb = const.tile([128, kc], F32, name="mneg_sb")
            nc.sync.dma_start(out=mneg_sb,
                              in_=maskneg.rearrange("(t p) -> p t", p=128))
            eps_sb = const.tile([128, 1], F32, name="eps_sb")
            nc.vector.memset(eps_sb, EPS)
            # invocation-constant inputs: loaded once, resident in SBUF
            xT_sb = const.tile([128, DT, S], FP8, name="xT_sb")
            nc.sync.dma_start(out=xT_sb, in_=xT8)
            xcT_sb = const.tile([128, DT, SC], FP8, name="xcT_sb")
            nc.sync.dma_start(out=xcT_sb, in_=xcT8)
            wq_sb = const.tile([128, DT, D], FP8, name="wq_sb")
            nc.sync.dma_start(out=wq_sb, in_=wq8)
            wk_sb = const.tile([128, DT, D], FP8, name="wk_sb")
            nc.sync.dma_start(out=wk_sb, in_=wk8)
            wv_sb = const.tile([128, DT, D], FP8, name="wv_sb")
            nc.sync.dma_start(out=wv_sb, in_=wv8)
            wo_sb = const.tile([128, DT, D], FP8, name="wo_sb")
            nc.sync.dma_start(out=wo_sb, in_=wo8)
            x_sb = const.tile([128, ST, D], F32, name="x_sb")
            nc.sync.dma_start(out=x_sb,
                              in_=x32.rearrange("(t p) d -> p t d", p=128))
            qT = const.tile([128, DT, S], FP8, name="qT")
            kT = const.tile([128, DT, SC], FP8, name="kT")
            vp = const.tile([128, kc + 1, H, VST], FP8, name="vp")
            ctxT = const.tile([128, DT, S], FP8, name="ctxT")
            # ones columns of v' (col 64 of each head slot): body movers only
            # touch cols 0..63, so write these once
            nc.vector.memset(vp.rearrange("p t h j -> p (t h) j")[:, :, 64],
                             1.0)
            # zeroed pad key-plane (must come after the ones memset): lets
            # the odd-kc AV tail run DoubleRow with a garbage E plane
            nc.vector.memset(vp[:, kc, :, :], 0.0)
            cst = (mneg_sb, eps_sb, qT, kT, vp, ctxT, xT_sb, xcT_sb,
                   wq_sb, wk_sb, wv_sb, wo_sb, x_sb)
            for _ in range(reps):
                _emit_body(nc, tc, io, cst, kc)

    nc.compile()
    return nc


_NC_CACHE = {}


def _get_nc(reps=1, kc=8):
    if (reps, kc) not in _NC_CACHE:
        _NC_CACHE[(reps, kc)] = build_bass(reps, kc)
    return _NC_CACHE[(reps, kc)]


def _pack_w(w):
    # [D, D] -> [128, DT, D] fp8 with w8[p, t, n] = w[t*128+p, n] * WS
    return np.ascontiguousarray(
        (np.asarray(w, np.float32) * WS).reshape(DT, 128, D)
        .transpose(1, 0, 2)).astype(ml_dtypes.float8_e4m3)


def _pack_xT(x):
    # [S', D] -> [128, DT, S'] fp8 with xT8[p, t, s] = x[s, t*128+p]
    return np.ascontiguousarray(
        np.asarray(x, np.float32).T.reshape(DT, 128, -1)
        .transpose(1, 0, 2)).astype(ml_dtypes.float8_e4m3)


def make_in_maps(x, mask, wq, bq, wk, bk, wv, bv, wo, bo, gamma, beta):
    for b in (bq, bk, bv, bo):
        assert not np.any(np.asarray(b)), "nonzero bias unsupported"
    x = np.asarray(x, dtype=np.float32)
    mask = np.asarray(mask)
    maskneg = (mask.astype(np.float32) * NINF).astype(np.float32)
    n_un = int((mask == 0).sum(axis=1).max())
    kc = min(max((n_un + 127) // 128, 2), ST)
    SC = kc * 128
    idxs = [np.argsort(mask[c], kind="stable")[:SC] for c in range(B)]
    common = {
        "wq8": _pack_w(wq), "wk8": _pack_w(wk),
        "wv8": _pack_w(wv), "wo8": _pack_w(wo),
    }
    maps = []
    for c in range(B):
        xc = x[c][idxs[c]]
        maps.append(dict(
            common,
            x32=np.ascontiguousarray(x[c]),
            xT8=_pack_xT(x[c]),
            xcT8=_pack_xT(xc),
            maskneg=np.ascontiguousarray(maskneg[c][idxs[c]])))
    return maps, kc


def kernel(x, mask, wq, bq, wk, bk, wv, bv, wo, bo, gamma, beta):
    in_maps, kc = make_in_maps(x, mask, wq, bq, wk, bk, wv, bv, wo, bo,
                               gamma, beta)
    nc = _get_nc(1, kc)
    last_err = None
    for _ in range(3):
        try:
            res = run_bass_kernel_spmd(nc, in_maps, core_ids=list(range(B)))
            outv = np.stack([res.results[c]["out"] for c in range(B)], axis=0)
            gamma = np.asarray(gamma, np.float32)
            beta = np.asarray(beta, np.float32)
            if not (np.all(gamma == 1.0) and np.all(beta == 0.0)):
                outv = outv * gamma + beta
            return outv
        except Exception as e:  # transient NRT device errors: retry
            last_err = e
            time.sleep(5)
    raise last_err


# revision 14
# speedup vs baseline: 1.0138x; 1.0010x over previous
"""Multi-head attention block on 8 Trainium2 NeuronCores, data-parallel over
batch, fp8 (e4m3) matmul datapath with DoubleRow perf mode.

Shapes (hardcoded): B=8, S=1024, H=16, HD=64, D=1024. One batch element per
core. Host pre-transposes/casts x and the weights to fp8 (weights scaled by
64 for fp8 range; the 1/64 is folded into the PSUM movers), compacts keys
(unmasked first), and applies gamma/beta after gathering (they are identity
for the spec inputs). Biases are zero per the input spec and asserted so.

Per-core dataflow:
  phase 1: qT/kT (fp8, [128d-of-dt, DT, S|SC]) and v' ([128keys, kc, H, 72]
           with a ones column per head at col 64) via DoubleRow fp8 matmuls
           over host-packed xT8/xcT8/w8; DVE tensor_scalar movers apply 1/64.
  phase 2 per head: scoresT[k,q] psum <- kT-slice^T @ qT (K=64);
           E = exp(0.125*scores + maskneg_k) via one ACT per kt -> fp8 SBUF;
           ctx' psum [65, 2, 512] accumulates v'^T @ E DoubleRow over kt
           pairs (rows 0..63 ctx, row 64 = colsum = softmax denominator);
           denom reciprocal via one int32 Schraudolph tensor_scalar on the
           psum row, gpsimd partition_broadcast, then one fused TT that
           normalizes and casts ctx -> fp8 ctxT.
  phase 3: out proj fp8 DoubleRow (ctxT @ wo8), epilogue
           t = x + proj/64, LayerNorm via bn_stats/bn_aggr + sqrt + recip.
"""
import sys
import time

sys.path.insert(0, "/opt/trn_rl_repo")

import numpy as np
import ml_dtypes

import concourse.bass as bass
import concourse.bacc as bacc
import concourse.tile as tile
from concourse import mybir
from concourse.bass_utils import run_bass_kernel_spmd

F32 = mybir.dt.float32
FP8 = mybir.dt.float8e4
INT32 = mybir.dt.int32
AF = mybir.ActivationFunctionType
DR = mybir.MatmulPerfMode.DoubleRow
MUL = mybir.AluOpType.mult
ADD = mybir.AluOpType.add

B, S, H, HD = 8, 1024, 16, 64
D = H * HD
NINF = -10000.0
EPS = 1e-6
ST = S // 128    # 8 s-tiles
DT = D // 128    # 8 d-tiles
NH = S // 512    # 2 query halves
WS = 64.0        # weight scale for fp8
IWS = 1.0 / WS
VST = 72         # per-head stride in v' (64 v dims + ones col + pad)
RECIP_C = 0x7EF311C3  # Schraudolph reciprocal constant


def _emit_body(nc, tc, io, cst, kc):
    (x32, out) = io
    (mneg_sb, eps_sb, qT, kT, vp, ctxT, xT_sb, xcT_sb,
     wq_sb, wk_sb, wv_sb, wo_sb, x_sb) = cst
    SC = kc * 128
    kpairs = kc // 2          # full DoubleRow key-tile pairs for AV
    ktail = kc - 2 * kpairs   # 0 or 1 leftover key tile

    if True:
        # ---------------- phases 1+2 merged: projections feed heads ------
        # V first, then per dt: Q/K projections for that d-group followed by
        # its two heads. The projection matmuls keep the PE busy during the
        # exp (ACT) waits of the attention chain, so HAM stays warm.
        with (
            tc.tile_pool(name="Ep", bufs=4) as Ep,
            tc.tile_pool(name="nrm", bufs=4) as nrm,
            tc.tile_pool(name="ps1", bufs=1, space="PSUM") as ps1,
            tc.tile_pool(name="scps", bufs=2, space="PSUM") as scps,
            tc.tile_pool(name="ctxps", bufs=3, space="PSUM") as ctxps,
        ):
            # v' natural [keys, dcols], scattered to head slots
            for st in range(kc):
                for et in range(2):
                    vps = ps1.tile([128, 512], F32, tag="qps", name="vps")
                    for kp in range(0, DT, 2):
                        nc.tensor.matmul(
                            vps,
                            xcT_sb[:, kp:kp + 2, st * 128:(st + 1) * 128],
                            wv_sb[:, kp:kp + 2, et * 512:(et + 1) * 512],
                            start=(kp == 0), stop=(kp == DT - 2),
                            perf_mode=DR)
                    nc.vector.tensor_scalar(
                        vp[:, st, et * 8:(et + 1) * 8, 0:64],
                        vps.rearrange("p (h j) -> p h j", j=64),
                        IWS, None, op0=MUL)

            k_tiles = [(i * 512, min(512, SC - i * 512))
                       for i in range((SC + 511) // 512)]
            q_tiles = [(i * 512, 512) for i in range(NH)]

            def project(wsb_t, dstT, rhsT, mg, ntiles):
                for n0, nsz in ntiles:
                    qps = ps1.tile([128, 512], F32, tag="qps", name="qps")
                    for kp in range(0, DT, 2):
                        nc.tensor.matmul(
                            qps[:, 0:nsz],
                            wsb_t[:, kp:kp + 2, mg * 128:(mg + 1) * 128],
                            rhsT[:, kp:kp + 2, n0:n0 + nsz],
                            start=(kp == 0), stop=(kp == DT - 2),
                            perf_mode=DR)
                    nc.vector.tensor_scalar(
                        dstT[:, mg, n0:n0 + nsz], qps[:, 0:nsz],
                        IWS, None, op0=MUL)

            def head(h):
                base = (h % 2) * 64
                dt = h // 2
                cps = [ctxps.tile([65, 512], F32, tag="ctx", name="cps")
                       for _ in range(NH)]
                e_pair = None
                for kt in range(kc):
                    scp = scps.tile([128, 2, 512], F32, tag="scp",
                                    name="scp")
                    for qh in range(NH):
                        nc.tensor.matmul(
                            scp[:, qh, :],
                            kT[base:base + 64, dt,
                               kt * 128:(kt + 1) * 128],
                            qT[base:base + 64, dt,
                               qh * 512:(qh + 1) * 512],
                            start=True, stop=True)
                    if kt % 2 == 0:
                        e_pair = Ep.tile([128, 2, 2, 512], FP8, tag="E",
                                         name="e_pair")
                    nc.scalar.activation(
                        e_pair[:, kt % 2, :, :], scp, AF.Exp,
                        bias=mneg_sb[:, kt:kt + 1], scale=0.125)
                    last_pair = (kt % 2 == 1 and ktail == 0
                                 and kt == kc - 1)
                    if kt % 2 == 1:
                        for qh in range(NH):
                            nc.tensor.matmul(
                                cps[qh],
                                vp[:, kt - 1:kt + 1, h, 0:65],
                                e_pair[:, 0:2, qh, :],
                                start=(kt == 1), stop=last_pair,
                                perf_mode=DR)
                    elif kt == kc - 1:
                        # odd kc tail: DoubleRow against the zeroed pad
                        # plane of vp. The moving pair dim has stride 0 so
                        # plane 1 re-reads the freshly written plane 0 --
                        # never stale SBUF (uninitialized e4m3 can be NaN,
                        # and 0 * NaN = NaN even against zero weights).
                        for qh in range(NH):
                            e0 = e_pair[:, 0, qh, :]
                            e00 = bass.AP(
                                tensor=e0.tensor, offset=e0.offset,
                                ap=[list(e0.ap[0]), [0, 2]]
                                + [list(a) for a in e0.ap[1:]])
                            nc.tensor.matmul(
                                cps[qh],
                                vp[:, kt:kt + 2, h, 0:65],
                                e00,
                                start=(kc == 1), stop=True,
                                perf_mode=DR)

                # softmax denominators: Schraudolph reciprocal of psum row 64
                for qh in range(NH):
                    r0 = nrm.tile([1, 512], F32, tag=f"r0{qh}", name="r0")
                    nc.vector.tensor_scalar(
                        r0.bitcast(INT32), cps[qh][64:65, :].bitcast(INT32),
                        -1, RECIP_C, op0=MUL, op1=ADD)
                    rbc = nrm.tile([64, 512], F32, tag=f"rbc{qh}",
                                   name="rbc")
                    nc.gpsimd.partition_broadcast(rbc, r0)
                    # fused normalize + psum drain + fp8 cast
                    nc.vector.tensor_mul(
                        ctxT[base:base + 64, dt,
                             qh * 512:(qh + 1) * 512],
                        cps[qh][0:64, :], rbc)

            for dt in range(DT):
                project(wq_sb, qT, xT_sb, dt, q_tiles)
                project(wk_sb, kT, xcT_sb, dt, k_tiles)
                head(2 * dt)
                head(2 * dt + 1)

        # ---------------- phase 3: out proj + LayerNorm ----------------
        with (
            tc.tile_pool(name="epi", bufs=3) as epi,
            tc.tile_pool(name="pjps", bufs=2, space="PSUM") as pjps,
        ):
            for qt in range(ST):
                x_t = x_sb[:, qt, :]
                t = epi.tile([128, D], F32, tag="t", name="t")
                for et in range(2):
                    pps = pjps.tile([128, 512], F32, tag=f"pj{et}",
                                    name="pps")
                    for dp in range(0, DT, 2):
                        nc.tensor.matmul(
                            pps,
                            ctxT[:, dp:dp + 2, qt * 128:(qt + 1) * 128],
                            wo_sb[:, dp:dp + 2, et * 512:(et + 1) * 512],
                            start=(dp == 0), stop=(dp == DT - 2),
                            perf_mode=DR)
                    nc.vector.scalar_tensor_tensor(
                        out=t[:, et * 512:(et + 1) * 512],
                        in0=pps, scalar=IWS,
                        in1=x_t[:, et * 512:(et + 1) * 512],
                        op0=MUL, op1=ADD)
                stats = epi.tile([128, 2, nc.vector.BN_STATS_DIM], F32,
                                 tag="stats", name="stats")
                tg = t.rearrange("p (g d) -> p g d", g=2)
                for g in range(2):
                    nc.vector.bn_stats(stats[:, g, :], tg[:, g, :])
                mv = epi.tile([128, nc.vector.BN_AGGR_DIM], F32, tag="mv",
                              name="mv")
                nc.vector.bn_aggr(mv, stats)
                rstd = epi.tile([128, 1], F32, tag="rstd", name="rstd")
                nc.scalar.activation(
                    rstd, mv[:, 1:2], AF.Sqrt, bias=eps_sb, scale=1.0)
                nc.vector.reciprocal(rstd, rstd)
                nmr = epi.tile([128, 1], F32, tag="nmr", name="nmr")
                nc.vector.tensor_scalar(
                    nmr, mv[:, 0:1], rstd, -1.0, op0=MUL, op1=MUL)
                o_t = epi.tile([128, D], F32, tag="ot", name="o_t")
                nc.scalar.activation(
                    o_t, t, AF.Identity, bias=nmr, scale=rstd)
                nc.sync.dma_start(
                    out=out[qt * 128:(qt + 1) * 128, :], in_=o_t)


def build_bass(reps=1, kc=8):
    nc = bacc.Bacc("TRN2", target_bir_lowering=False, debug=False)

    SC = kc * 128
    x32 = nc.dram_tensor("x32", [S, D], F32, kind="ExternalInput").ap()
    xT8 = nc.dram_tensor("xT8", [128, DT, S], FP8, kind="ExternalInput").ap()
    xcT8 = nc.dram_tensor("xcT8", [128, DT, SC], FP8,
                          kind="ExternalInput").ap()
    wq8 = nc.dram_tensor("wq8", [128, DT, D], FP8, kind="ExternalInput").ap()
    wk8 = nc.dram_tensor("wk8", [128, DT, D], FP8, kind="ExternalInput").ap()
    wv8 = nc.dram_tensor("wv8", [128, DT, D], FP8, kind="ExternalInput").ap()
    wo8 = nc.dram_tensor("wo8", [128, DT, D], FP8, kind="ExternalInput").ap()
    maskneg = nc.dram_tensor("maskneg", [SC], F32, kind="ExternalInput").ap()
    out = nc.dram_tensor("out", [S, D], F32, kind="ExternalOutput").ap()
    io = (x32, out)

    with tile.TileContext(nc) as tc:
        with tc.tile_pool(name="const", bufs=1) as const:
            mneg_sb = const.tile([128, kc], F32, name="mneg_sb")
            nc.sync.dma_start(out=mneg_sb,
                              in_=maskneg.rearrange("(t p) -> p t", p=128))
            eps_sb = const.tile([128, 1], F32, name="eps_sb")
            nc.vector.memset(eps_sb, EPS)
            # invocation-constant inputs: loaded once, resident in SBUF
            xT_sb = const.tile([128, DT, S], FP8, name="xT_sb")
            nc.sync.dma_start(out=xT_sb, in_=xT8)
            xcT_sb = const.tile([128, DT, SC], FP8, name="xcT_sb")
            nc.sync.dma_start(out=xcT_sb, in_=xcT8)
            wq_sb = const.tile([128, DT, D], FP8, name="wq_sb")
            nc.sync.dma_start(out=wq_sb, in_=wq8)
            wk_sb = const.tile([128, DT, D], FP8, name="wk_sb")
            nc.sync.dma_start(out=wk_sb, in_=wk8)
            wv_sb = const.tile([128, DT, D], FP8, name="wv_sb")
            nc.sync.dma_start(out=wv_sb, in_=wv8)
            wo_sb = const.tile([128, DT, D], FP8, name="wo_sb")
            nc.sync.dma_start(out=wo_sb, in_=wo8)
            x_sb = const.tile([128, ST, D], F32, name="x_sb")
            nc.sync.dma_start(out=x_sb,
                              in_=x32.rearrange("(t p) d -> p t d", p=128))
            qT = const.tile([128, DT, S], FP8, name="qT")
            kT = const.tile([128, DT, SC], FP8, name="kT")
            vp = const.tile([128, kc + 1, H, VST], FP8, name="vp")
            ctxT = const.tile([128, DT, S], FP8, name="ctxT")
            # ones columns of v' (col 64 of each head slot): body movers only
            # touch cols 0..63, so write these once
            nc.vector.memset(vp.rearrange("p t h j -> p (t h) j")[:, :, 64],
                             1.0)
            # zeroed pad key-plane (must come after the ones memset): lets
            # the odd-kc AV tail run DoubleRow with a garbage E plane
            nc.vector.memset(vp[:, kc, :, :], 0.0)
            cst = (mneg_sb, eps_sb, qT, kT, vp, ctxT, xT_sb, xcT_sb,
                   wq_sb, wk_sb, wv_sb, wo_sb, x_sb)
            for _ in range(reps):
                _emit_body(nc, tc, io, cst, kc)

    nc.compile()
    return nc


_NC_CACHE = {}


def _get_nc(reps=1, kc=8):
    if (reps, kc) not in _NC_CACHE:
        _NC_CACHE[(reps, kc)] = build_bass(reps, kc)
    return _NC_CACHE[(reps, kc)]


def _pack_w(w):
    # [D, D] -> [128, DT, D] fp8 with w8[p, t, n] = w[t*128+p, n] * WS
    return np.ascontiguousarray(
        (np.asarray(w, np.float32) * WS).reshape(DT, 128, D)
        .transpose(1, 0, 2)).astype(ml_dtypes.float8_e4m3)


def _pack_xT(x):
    # [S', D] -> [128, DT, S'] fp8 with xT8[p, t, s] = x[s, t*128+p]
    return np.ascontiguousarray(
        np.asarray(x, np.float32).T.reshape(DT, 128, -1)
        .transpose(1, 0, 2)).astype(ml_dtypes.float8_e4m3)


def make_in_maps(x, mask, wq, bq, wk, bk, wv, bv, wo, bo, gamma, beta):
    for b in (bq, bk, bv, bo):
        assert not np.any(np.asarray(b)), "nonzero bias unsupported"
    x = np.asarray(x, dtype=np.float32)
    mask = np.asarray(mask)
    maskneg = (mask.astype(np.float32) * NINF).astype(np.float32)
    n_un = int((mask == 0).sum(axis=1).max())
    kc = min(max((n_un + 127) // 128, 2), ST)
    SC = kc * 128
    idxs = [np.argsort(mask[c], kind="stable")[:SC] for c in range(B)]
    common = {
        "wq8": _pack_w(wq), "wk8": _pack_w(wk),
        "wv8": _pack_w(wv), "wo8": _pack_w(wo),
    }
    maps = []
    for c in range(B):
        xc = x[c][idxs[c]]
        maps.append(dict(
            common,
            x32=np.ascontiguousarray(x[c]),
            xT8=_pack_xT(x[c]),
            xcT8=_pack_xT(xc),
            maskneg=np.ascontiguousarray(maskneg[c][idxs[c]])))
    return maps, kc


def kernel(x, mask, wq, bq, wk, bk, wv, bv, wo, bo, gamma, beta):
    in_maps, kc = make_in_maps(x, mask, wq, bq, wk, bk, wv, bv, wo, bo,
                               gamma, beta)
    nc = _get_nc(1, kc)
    last_err = None
    for _ in range(3):
        try:
            res = run_bass_kernel_spmd(nc, in_maps, core_ids=list(range(B)))
            outv = np.stack([res.results[c]["out"] for c in range(B)], axis=0)
            gamma = np.asarray(gamma, np.float32)
            beta = np.asarray(beta, np.float32)
            if not (np.all(gamma == 1.0) and np.all(beta == 0.0)):
                outv = outv * gamma + beta
            return outv
        except Exception as e:  # transient NRT device errors: retry
            last_err = e
            time.sleep(5)
    raise last_err


# revision 15
# speedup vs baseline: 1.0957x; 1.0808x over previous
"""Multi-head attention block on 8 Trainium2 NeuronCores, data-parallel over
batch, fp8 (e4m3) matmul datapath with DoubleRow perf mode.

Shapes (hardcoded): B=8, S=1024, H=16, HD=64, D=1024. One batch element per
core. Host pre-transposes/casts x and the weights to fp8 (weights scaled by
64 for fp8 range; the 1/64 is folded into the PSUM movers), compacts keys
(unmasked first), and applies gamma/beta after gathering (they are identity
for the spec inputs). Biases are zero per the input spec and asserted so.

Per-core dataflow:
  phase 1: qT/kT (fp8, [128d-of-dt, DT, S|SC]) and v' ([128keys, kc, H, 72]
           with a ones column per head at col 64) via DoubleRow fp8 matmuls
           over host-packed xT8/xcT8/w8; DVE tensor_scalar movers apply 1/64.
  phase 2 per head: scoresT[k,q] psum <- kT-slice^T @ qT (K=64);
           E = exp(0.125*scores + maskneg_k) via one ACT per kt -> fp8 SBUF;
           ctx' psum [65, 2, 512] accumulates v'^T @ E DoubleRow over kt
           pairs (rows 0..63 ctx, row 64 = colsum = softmax denominator);
           denom reciprocal via one int32 Schraudolph tensor_scalar on the
           psum row, gpsimd partition_broadcast, then one fused TT that
           normalizes and casts ctx -> fp8 ctxT.
  phase 3: out proj fp8 DoubleRow (ctxT @ wo8), epilogue
           t = x + proj/64, LayerNorm via bn_stats/bn_aggr + sqrt + recip.
"""
import sys
import time

sys.path.insert(0, "/opt/trn_rl_repo")

import numpy as np
import ml_dtypes

import concourse.bass as bass
import concourse.bacc as bacc
import concourse.tile as tile
from concourse import mybir
from concourse.bass_utils import run_bass_kernel_spmd

F32 = mybir.dt.float32
FP8 = mybir.dt.float8e4
INT32 = mybir.dt.int32
AF = mybir.ActivationFunctionType
DR = mybir.MatmulPerfMode.DoubleRow
MUL = mybir.AluOpType.mult
ADD = mybir.AluOpType.add

B, S, H, HD = 8, 1024, 16, 64
D = H * HD
NINF = -10000.0
EPS = 1e-6
ST = S // 128    # 8 s-tiles
DT = D // 128    # 8 d-tiles
NH = S // 512    # 2 query halves
WS = 64.0        # weight scale for fp8
IWS = 1.0 / WS
VST = 72         # per-head stride in v' (64 v dims + ones col + pad)
RECIP_C = 0x7EF311C3  # Schraudolph reciprocal constant


def _emit_body(nc, tc, io, cst, kc):
    (x32, out) = io
    (mneg_sb, eps_sb, qT, kT, vp, ctxT, xT_sb, xcT_sb,
     wq_sb, wk_sb, wv_sb, wo_sb, x_sb) = cst
    SC = kc * 128
    kpairs = kc // 2          # full DoubleRow key-tile pairs for AV
    ktail = kc - 2 * kpairs   # 0 or 1 leftover key tile

    if True:
        # ---------------- phases 1+2 merged: projections feed heads ------
        # V first, then per dt: Q/K projections for that d-group followed by
        # its two heads. The projection matmuls keep the PE busy during the
        # exp (ACT) waits of the attention chain, so HAM stays warm.
        with (
            tc.tile_pool(name="Ep", bufs=4) as Ep,
            tc.tile_pool(name="nrm", bufs=4) as nrm,
            tc.tile_pool(name="ps1", bufs=2, space="PSUM") as ps1,
            tc.tile_pool(name="scps", bufs=2, space="PSUM") as scps,
            tc.tile_pool(name="ctxps", bufs=2, space="PSUM") as ctxps,
        ):
            # v' natural [keys, dcols], scattered to head slots
            for st in range(kc):
                for et in range(2):
                    vps = ps1.tile([128, 512], F32, tag="qps", name="vps")
                    for kp in range(0, DT, 2):
                        nc.tensor.matmul(
                            vps,
                            xcT_sb[:, kp:kp + 2, st * 128:(st + 1) * 128],
                            wv_sb[:, kp:kp + 2, et * 512:(et + 1) * 512],
                            start=(kp == 0), stop=(kp == DT - 2),
                            perf_mode=DR)
                    nc.vector.tensor_scalar(
                        vp[:, st, et * 8:(et + 1) * 8, 0:64],
                        vps.rearrange("p (h j) -> p h j", j=64),
                        IWS, None, op0=MUL)

            k_tiles = [(i * 512, min(512, SC - i * 512))
                       for i in range((SC + 511) // 512)]
            q_tiles = [(i * 512, 512) for i in range(NH)]

            def project(wsb_t, dstT, rhsT, mg, ntiles):
                for n0, nsz in ntiles:
                    qps = ps1.tile([128, 512], F32, tag="qps", name="qps")
                    for kp in range(0, DT, 2):
                        nc.tensor.matmul(
                            qps[:, 0:nsz],
                            wsb_t[:, kp:kp + 2, mg * 128:(mg + 1) * 128],
                            rhsT[:, kp:kp + 2, n0:n0 + nsz],
                            start=(kp == 0), stop=(kp == DT - 2),
                            perf_mode=DR)
                    nc.vector.tensor_scalar(
                        dstT[:, mg, n0:n0 + nsz], qps[:, 0:nsz],
                        IWS, None, op0=MUL)

            def head(h):
                base = (h % 2) * 64
                dt = h // 2
                cps = [ctxps.tile([65, 512], F32, tag="ctx", name="cps")
                       for _ in range(NH)]
                e_pair = None
                for kt in range(kc):
                    scp = scps.tile([128, 2, 512], F32, tag="scp",
                                    name="scp")
                    for qh in range(NH):
                        nc.tensor.matmul(
                            scp[:, qh, :],
                            kT[base:base + 64, dt,
                               kt * 128:(kt + 1) * 128],
                            qT[base:base + 64, dt,
                               qh * 512:(qh + 1) * 512],
                            start=True, stop=True)
                    if kt % 2 == 0:
                        e_pair = Ep.tile([128, 2, 2, 512], FP8, tag="E",
                                         name="e_pair")
                    nc.scalar.activation(
                        e_pair[:, kt % 2, :, :], scp, AF.Exp,
                        bias=mneg_sb[:, kt:kt + 1], scale=0.125)
                    last_pair = (kt % 2 == 1 and ktail == 0
                                 and kt == kc - 1)
                    if kt % 2 == 1:
                        for qh in range(NH):
                            nc.tensor.matmul(
                                cps[qh],
                                vp[:, kt - 1:kt + 1, h, 0:65],
                                e_pair[:, 0:2, qh, :],
                                start=(kt == 1), stop=last_pair,
                                perf_mode=DR)
                    elif kt == kc - 1:
                        # odd kc tail: DoubleRow against the zeroed pad
                        # plane of vp. The moving pair dim has stride 0 so
                        # plane 1 re-reads the freshly written plane 0 --
                        # never stale SBUF (uninitialized e4m3 can be NaN,
                        # and 0 * NaN = NaN even against zero weights).
                        for qh in range(NH):
                            e0 = e_pair[:, 0, qh, :]
                            e00 = bass.AP(
                                tensor=e0.tensor, offset=e0.offset,
                                ap=[list(e0.ap[0]), [0, 2]]
                                + [list(a) for a in e0.ap[1:]])
                            nc.tensor.matmul(
                                cps[qh],
                                vp[:, kt:kt + 2, h, 0:65],
                                e00,
                                start=(kc == 1), stop=True,
                                perf_mode=DR)

                # softmax denominators: Schraudolph reciprocal of psum row 64
                for qh in range(NH):
                    r0 = nrm.tile([1, 512], F32, tag=f"r0{qh}", name="r0")
                    nc.vector.tensor_scalar(
                        r0.bitcast(INT32), cps[qh][64:65, :].bitcast(INT32),
                        -1, RECIP_C, op0=MUL, op1=ADD)
                    rbc = nrm.tile([64, 512], F32, tag=f"rbc{qh}",
                                   name="rbc")
                    nc.gpsimd.partition_broadcast(rbc, r0)
                    # fused normalize + psum drain + fp8 cast
                    nc.vector.tensor_mul(
                        ctxT[base:base + 64, dt,
                             qh * 512:(qh + 1) * 512],
                        cps[qh][0:64, :], rbc)

            for dt in range(DT):
                project(wq_sb, qT, xT_sb, dt, q_tiles)
                project(wk_sb, kT, xcT_sb, dt, k_tiles)
                head(2 * dt)
                head(2 * dt + 1)

        # ---------------- phase 3: out proj + LayerNorm ----------------
        with (
            tc.tile_pool(name="epi", bufs=3) as epi,
            tc.tile_pool(name="pjps", bufs=2, space="PSUM") as pjps,
        ):
            for qt in range(ST):
                x_t = x_sb[:, qt, :]
                t = epi.tile([128, D], F32, tag="t", name="t")
                for et in range(2):
                    pps = pjps.tile([128, 512], F32, tag=f"pj{et}",
                                    name="pps")
                    for dp in range(0, DT, 2):
                        nc.tensor.matmul(
                            pps,
                            ctxT[:, dp:dp + 2, qt * 128:(qt + 1) * 128],
                            wo_sb[:, dp:dp + 2, et * 512:(et + 1) * 512],
                            start=(dp == 0), stop=(dp == DT - 2),
                            perf_mode=DR)
                    nc.vector.scalar_tensor_tensor(
                        out=t[:, et * 512:(et + 1) * 512],
                        in0=pps, scalar=IWS,
                        in1=x_t[:, et * 512:(et + 1) * 512],
                        op0=MUL, op1=ADD)
                stats = epi.tile([128, 2, nc.vector.BN_STATS_DIM], F32,
                                 tag="stats", name="stats")
                tg = t.rearrange("p (g d) -> p g d", g=2)
                for g in range(2):
                    nc.vector.bn_stats(stats[:, g, :], tg[:, g, :])
                mv = epi.tile([128, nc.vector.BN_AGGR_DIM], F32, tag="mv",
                              name="mv")
                nc.vector.bn_aggr(mv, stats)
                rstd = epi.tile([128, 1], F32, tag="rstd", name="rstd")
                nc.scalar.activation(
                    rstd, mv[:, 1:2], AF.Sqrt, bias=eps_sb, scale=1.0)
                nc.vector.reciprocal(rstd, rstd)
                nmr = epi.tile([128, 1], F32, tag="nmr", name="nmr")
                nc.vector.tensor_scalar(
                    nmr, mv[:, 0:1], rstd, -1.0, op0=MUL, op1=MUL)
                o_t = epi.tile([128, D], F32, tag="ot", name="o_t")
                nc.scalar.activation(
                    o_t, t, AF.Identity, bias=nmr, scale=rstd)
                nc.sync.dma_start(
                    out=out[qt * 128:(qt + 1) * 128, :], in_=o_t)


def build_bass(reps=1, kc=8):
    nc = bacc.Bacc("TRN2", target_bir_lowering=False, debug=False)

    SC = kc * 128
    x32 = nc.dram_tensor("x32", [S, D], F32, kind="ExternalInput").ap()
    xT8 = nc.dram_tensor("xT8", [128, DT, S], FP8, kind="ExternalInput").ap()
    xcT8 = nc.dram_tensor("xcT8", [128, DT, SC], FP8,
                          kind="ExternalInput").ap()
    wq8 = nc.dram_tensor("wq8", [128, DT, D], FP8, kind="ExternalInput").ap()
    wk8 = nc.dram_tensor("wk8", [128, DT, D], FP8, kind="ExternalInput").ap()
    wv8 = nc.dram_tensor("wv8", [128, DT, D], FP8, kind="ExternalInput").ap()
    wo8 = nc.dram_tensor("wo8", [128, DT, D], FP8, kind="ExternalInput").ap()
    maskneg = nc.dram_tensor("maskneg", [SC], F32, kind="ExternalInput").ap()
    out = nc.dram_tensor("out", [S, D], F32, kind="ExternalOutput").ap()
    io = (x32, out)

    with tile.TileContext(nc) as tc:
        with tc.tile_pool(name="const", bufs=1) as const:
            mneg_sb = const.tile([128, kc], F32, name="mneg_sb")
            nc.sync.dma_start(out=mneg_sb,
                              in_=maskneg.rearrange("(t p) -> p t", p=128))
            eps_sb = const.tile([128, 1], F32, name="eps_sb")
            nc.vector.memset(eps_sb, EPS)
            # invocation-constant inputs: loaded once, resident in SBUF
            xT_sb = const.tile([128, DT, S], FP8, name="xT_sb")
            nc.sync.dma_start(out=xT_sb, in_=xT8)
            xcT_sb = const.tile([128, DT, SC], FP8, name="xcT_sb")
            nc.sync.dma_start(out=xcT_sb, in_=xcT8)
            wq_sb = const.tile([128, DT, D], FP8, name="wq_sb")
            nc.sync.dma_start(out=wq_sb, in_=wq8)
            wk_sb = const.tile([128, DT, D], FP8, name="wk_sb")
            nc.sync.dma_start(out=wk_sb, in_=wk8)
            wv_sb = const.tile([128, DT, D], FP8, name="wv_sb")
            nc.sync.dma_start(out=wv_sb, in_=wv8)
            wo_sb = const.tile([128, DT, D], FP8, name="wo_sb")
            nc.sync.dma_start(out=wo_sb, in_=wo8)
            x_sb = const.tile([128, ST, D], F32, name="x_sb")
            nc.sync.dma_start(out=x_sb,
                              in_=x32.rearrange("(t p) d -> p t d", p=128))
            qT = const.tile([128, DT, S], FP8, name="qT")
            kT = const.tile([128, DT, SC], FP8, name="kT")
            vp = const.tile([128, kc + 1, H, VST], FP8, name="vp")
            ctxT = const.tile([128, DT, S], FP8, name="ctxT")
            # ones columns of v' (col 64 of each head slot): body movers only
            # touch cols 0..63, so write these once
            nc.vector.memset(vp.rearrange("p t h j -> p (t h) j")[:, :, 64],
                             1.0)
            # zeroed pad key-plane (must come after the ones memset): lets
            # the odd-kc AV tail run DoubleRow with a garbage E plane
            nc.vector.memset(vp[:, kc, :, :], 0.0)
            cst = (mneg_sb, eps_sb, qT, kT, vp, ctxT, xT_sb, xcT_sb,
                   wq_sb, wk_sb, wv_sb, wo_sb, x_sb)
            for _ in range(reps):
                _emit_body(nc, tc, io, cst, kc)

    nc.compile()
    return nc


_NC_CACHE = {}


def _get_nc(reps=1, kc=8):
    if (reps, kc) not in _NC_CACHE:
        _NC_CACHE[(reps, kc)] = build_bass(reps, kc)
    return _NC_CACHE[(reps, kc)]


def _pack_w(w):
    # [D, D] -> [128, DT, D] fp8 with w8[p, t, n] = w[t*128+p, n] * WS
    return np.ascontiguousarray(
        (np.asarray(w, np.float32) * WS).reshape(DT, 128, D)
        .transpose(1, 0, 2)).astype(ml_dtypes.float8_e4m3)


def _pack_xT(x):
    # [S', D] -> [128, DT, S'] fp8 with xT8[p, t, s] = x[s, t*128+p]
    return np.ascontiguousarray(
        np.asarray(x, np.float32).T.reshape(DT, 128, -1)
        .transpose(1, 0, 2)).astype(ml_dtypes.float8_e4m3)


def make_in_maps(x, mask, wq, bq, wk, bk, wv, bv, wo, bo, gamma, beta):
    for b in (bq, bk, bv, bo):
        assert not np.any(np.asarray(b)), "nonzero bias unsupported"
    x = np.asarray(x, dtype=np.float32)
    mask = np.asarray(mask)
    maskneg = (mask.astype(np.float32) * NINF).astype(np.float32)
    n_un = int((mask == 0).sum(axis=1).max())
    kc = min(max((n_un + 127) // 128, 2), ST)
    SC = kc * 128
    idxs = [np.argsort(mask[c], kind="stable")[:SC] for c in range(B)]
    common = {
        "wq8": _pack_w(wq), "wk8": _pack_w(wk),
        "wv8": _pack_w(wv), "wo8": _pack_w(wo),
    }
    maps = []
    for c in range(B):
        xc = x[c][idxs[c]]
        maps.append(dict(
            common,
            x32=np.ascontiguousarray(x[c]),
            xT8=_pack_xT(x[c]),
            xcT8=_pack_xT(xc),
            maskneg=np.ascontiguousarray(maskneg[c][idxs[c]])))
    return maps, kc


def kernel(x, mask, wq, bq, wk, bk, wv, bv, wo, bo, gamma, beta):
    in_maps, kc = make_in_maps(x, mask, wq, bq, wk, bk, wv, bv, wo, bo,
                               gamma, beta)
    nc = _get_nc(1, kc)
    last_err = None
    for _ in range(3):
        try:
            res = run_bass_kernel_spmd(nc, in_maps, core_ids=list(range(B)))
            outv = np.stack([res.results[c]["out"] for c in range(B)], axis=0)
            gamma = np.asarray(gamma, np.float32)
            beta = np.asarray(beta, np.float32)
            if not (np.all(gamma == 1.0) and np.all(beta == 0.0)):
                outv = outv * gamma + beta
            return outv
        except Exception as e:  # transient NRT device errors: retry
            last_err = e
            time.sleep(5)
    raise last_err
